# revision 20
# baseline (speedup 1.0000x reference)
"""Trainium2 Bass kernel for nn_LogisticMixture (discretized logistic mixture loss).

Contract: kernel(**inputs) takes FULL unsharded numpy inputs
  x      [128, 32, 32, 256] f32
  value  [128, 32, 32, 3]   f32 (integer pixel values 0..255)
  W_conv [256, 100]         f32
  b_conv [100]              f32
and returns the full [128] f32 output (per-image sum of mixture log-probs).

Strategy: pure data parallelism over batch across 8 NeuronCores (16384
pixels/core). Host pre-transposes x to x^T [256, 16384] bf16 per core and
PRESCALES W columns (locs *= -127.5, coeffs *= -1) so the PE emits
epilogue-ready params. Epilogue per 2048-px macro-tile, software-pipelined
8 stages deep so every cross-engine dependency is >=1 tile old:

  A_c   = vp2_c + locs'_c (+ q*coeff' couplings)        [vp2 = value-127]
  sct   = 127.5*ln(1+e^s) + 127.5e-7 ; r = 1/sct        (recip on DVE or
          via ACT exp(-ln(sct)), split per-tile to balance engines)
  p0 = A*r ; nm0 = r - p0  (negated min_in)
  masked-shift: +30000 host tensors kill u/v/L2 terms for y==0/255 exactly
  u+v   = L01 - min(pm',0) with L01 = ln(1+e^{-|pm'|})  (one paired op)
  w     = u + v - ln(1-e^{-r'}) ; S = sum_c w
  mix   = lse_k(logits - S) - lse_k(logits)             -> acc per image

Engine split: DVE does the f16 2x/4x-mode elementwise; ACT all exp/ln
(single natural_log_exp_and_others table); GpSimd (Pool) the coupling
products, psum copies and f32 adds; PE the 1x1-conv matmuls.
"""
import sys
import os

for _p in ("/opt/trn_rl_repo", "/root/.axon_site/_ro/trn_rl_repo"):
    if os.path.isdir(_p) and _p not in sys.path:
        sys.path.append(_p)

import numpy as np
import ml_dtypes

import concourse.bass as bass
import concourse.mybir as mybir
import concourse.tile as tile
from concourse import bacc
from concourse.bass_utils import run_bass_kernel_spmd
import concourse.hw_specs as hw_specs

F32 = mybir.dt.float32
F16 = mybir.dt.float16
BF16 = mybir.dt.bfloat16
AL = mybir.AluOpType
AF = mybir.ActivationFunctionType

N_CORES = 8
D = 256
M = 100          # NUM_MIX * NUM_OUT
K = 10           # mixtures
C = 3            # channels
C2 = float(127.5 * np.exp(-7.0))   # eps folded into scales_t
KBIG = 30000.0                     # masked-shift magnitude (f16-exact)
N_ACT_RECIP = 3                    # tiles whose reciprocal runs on ACT


def _force_single_act_table():
    """All ACT funcs used here (Exp, Ln) live in natural_log_exp_and_others.
    The default chooser flip-flops between exp/ln sets, reloading tables
    (~1.3us each). Empty every other set (keeping dict order so set ids stay
    aligned with act_info.json) so one table load serves the whole kernel."""
    if getattr(hw_specs, "_ant_single_set", False):
        return
    orig = hw_specs.get_activation_tables
    import functools

    @functools.cache
    def patched(arch):
        tabs = dict(orig(arch))
        keep = "natural_log_exp_and_others"
        if keep in tabs:
            tabs = {k: (v if k == keep else set()) for k, v in tabs.items()}
        return tabs

    hw_specs.get_activation_tables = patched
    bacc.get_activation_tables = patched
    hw_specs._ant_single_set = True


def _v(ap0, offset, pattern):
    """AP on a tile's [:, :] AP: keep partition dim, replace free dims
    (stride-0 broadcast dims allowed)."""
    if not isinstance(ap0, bass.AP):
        ap0 = ap0[:, :]
    return bass.AP(tensor=ap0.tensor, offset=ap0.offset + offset,
                   ap=[list(ap0.ap[0])] + [list(p) for p in pattern])


def build_program(pix=16384, with_bias=False):
    """Single-core SPMD program. pix must be a multiple of 2048."""
    TP = 2048                  # pixels per macro-tile (2 images)
    NT = pix // TP             # macro-tiles
    NS = 16                    # 128-px subtiles per macro-tile
    NIMG = pix // 1024
    NKC = NS * K * C           # 480
    NK = NS * K                # 160

    _force_single_act_table()
    nc = bacc.Bacc("TRN2", target_bir_lowering=False, debug=False)

    xT_d = nc.dram_tensor("xT", [NT * D, TP], BF16, kind="ExternalInput").ap()
    w_d = nc.dram_tensor("w", [D, M], BF16, kind="ExternalInput").ap()
    vp_d = nc.dram_tensor("vp", [128, NT * NS * C], F16, kind="ExternalInput").ap()
    q_d = nc.dram_tensor("q", [128, NT * NS * 2], F16, kind="ExternalInput").ap()
    aux_d = nc.dram_tensor("aux", [128, NT * 3 * NKC], F16,
                           kind="ExternalInput").ap()
    if with_bias:
        bias_d = nc.dram_tensor("bias", [1, M], BF16, kind="ExternalInput").ap()
    acc_d = nc.dram_tensor("acc", [128, NIMG], F32, kind="ExternalOutput").ap()

    with tile.TileContext(nc) as tc, \
            tc.tile_pool(name="const", bufs=1) as cpool, \
            tc.tile_pool(name="xin", bufs=3) as xpool, \
            tc.tile_pool(name="ps", bufs=2, space="PSUM") as pspool, \
            tc.tile_pool(name="ep", bufs=3) as ep, \
            tc.tile_pool(name="ep4", bufs=4) as ep4, \
            tc.tile_pool(name="ep5", bufs=5) as ep5:

        # DMA order matters for pipeline fill: w + first x tiles gate the
        # PE; aux is not needed until s2(0), so it goes last.
        w_sb = cpool.tile([128, 2 * M], BF16)
        nc.sync.dma_start(out=w_sb, in_=bass.AP(
            tensor=w_d.tensor, offset=0, ap=[[M, 128], [128 * M, 2], [1, M]]))
        vp_sb = cpool.tile([128, NT * NS * C], F16)
        q_sb = cpool.tile([128, NT * NS * 2], F16)
        aux_sb = cpool.tile([128, NT * 3 * NKC], F16)
        acc = cpool.tile([128, NIMG], F32)
        zero_sb = cpool.tile([128, NKC], F32)
        nc.gpsimd.memset(zero_sb, 0.0)
        if with_bias:
            bias_sb = cpool.tile([1, M], BF16)
            nc.sync.dma_start(out=bias_sb, in_=bias_d)
            ones_sb = cpool.tile([1, 128], BF16)
            nc.vector.memset(ones_sb, 1.0)

        # per-tile state passed between stages
        st = [dict() for _ in range(NT)]

        # AP dim patterns
        #  psum field view for field f0 at offset f: [s(128), k(10), (c)]
        def fview(ps, f, cdim=False):
            dims = [[128, NS], [K, K]] + ([[1, C]] if cdim else [])
            return _v(ps[:, :], f, dims)

        cb_out = [[K, NS], [1, K], [NK, C]]     # c-blocked out iter [s,k,c]

        def dma_tile(j):
            xt0 = xpool.tile([128, TP], BF16, tag="xt0")
            xt1 = xpool.tile([128, TP], BF16, tag="xt1")
            nc.sync.dma_start(out=xt0, in_=xT_d[j * D:j * D + 128, :])
            nc.sync.dma_start(out=xt1, in_=xT_d[j * D + 128:(j + 1) * D, :])
            st[j]["xt"] = (xt0, xt1)

        def s1(j):
            xt0, xt1 = st[j].pop("xt")
            ps = pspool.tile([128, 2048], F32, tag="ps")
            for sub in range(NS):
                o = ps[:, sub * 128:sub * 128 + M]
                if with_bias:
                    nc.tensor.matmul(o, ones_sb[:, :], bias_sb[:, :],
                                     start=True, stop=False)
                nc.tensor.matmul(o, xt0[:, sub * 128:(sub + 1) * 128],
                                 w_sb[:, 0:M], start=not with_bias, stop=False)
                nc.tensor.matmul(o, xt1[:, sub * 128:(sub + 1) * 128],
                                 w_sb[:, M:2 * M], start=False, stop=True)

            # ACT: Es = exp(s_raw) (c-blocked out), Ls = ln(1+Es)
            es = ep.tile([128, NKC], F16, tag="es")
            nc.scalar.activation(_v(es[:, :], 0, cb_out), fview(ps, 4, True),
                                 AF.Exp)
            ls = ep.tile([128, NKC], F16, tag="ls")
            nc.scalar.activation(ls, es, AF.Ln, bias=1.0)
            # ACT: e2t = exp(logits) -> epair[160:320]
            epair = ep5.tile([128, 2 * NK], F32, tag="epair")
            nc.scalar.activation(epair[:, NK:2 * NK], fview(ps, 0), AF.Exp)
            # ACT copies out of PSUM (GpSimd cannot touch PSUM on TRN2):
            # logits f0 -> f0s, coeff fields f7..9 -> cc (coeff-blocked)
            f0s = ep4.tile([128, NK], F32, tag="f0s")
            nc.scalar.activation(f0s, fview(ps, 0), AF.Copy)
            cc = ep.tile([128, NKC], F16, tag="cc")
            nc.scalar.activation(_v(cc[:, :], 0, cb_out), fview(ps, 7, True),
                                 AF.Copy)
            # DVE: A = locs' + vp2 (c-blocked)
            A = ep.tile([128, NKC], F16, tag="A")
            vpb = _v(vp_sb[:, :], j * NS * C, [[C, NS], [0, K], [1, C]])
            nc.vector.tensor_tensor(_v(A[:, :], 0, cb_out), fview(ps, 1, True),
                                    vpb, AL.add)
            # Pool: coupling products + adds into A (all SBUF)
            qb0 = _v(q_sb[:, :], j * NS * 2, [[2, NS], [0, K]])
            qb1 = _v(q_sb[:, :], j * NS * 2 + 1, [[2, NS], [0, K]])
            h0 = ep.tile([128, NK], F16, tag="h0")
            h1 = ep.tile([128, NK], F16, tag="h1")
            h2 = ep.tile([128, NK], F16, tag="h2")
            nc.gpsimd.tensor_tensor(h0, qb0, cc[:, 0:NK], AL.mult)
            nc.gpsimd.tensor_tensor(h1, qb0, cc[:, NK:2 * NK], AL.mult)
            nc.gpsimd.tensor_tensor(h2, qb1, cc[:, 2 * NK:3 * NK], AL.mult)
            nc.gpsimd.tensor_tensor(A[:, NK:2 * NK], A[:, NK:2 * NK], h0,
                                    AL.add)
            nc.gpsimd.tensor_tensor(A[:, 2 * NK:3 * NK], A[:, 2 * NK:3 * NK],
                                    h1, AL.add)
            nc.gpsimd.tensor_tensor(A[:, 2 * NK:3 * NK], A[:, 2 * NK:3 * NK],
                                    h2, AL.add)
            st[j]["es"] = es
            st[j]["ls"] = ls
            st[j]["epair"] = epair
            st[j]["f0s"] = f0s
            st[j]["A"] = A

        def s1b(j):
            ls = st[j].pop("ls")
            sct = ep.tile([128, NKC], F32, tag="sct")
            nc.vector.tensor_scalar(sct, ls, 127.5, C2, AL.mult, AL.add)
            triple = ep.tile([128, 3 * NKC], F16, tag="triple")
            if j < N_ACT_RECIP:
                lr = ep.tile([128, NKC], F32, tag="lr")
                nc.scalar.activation(lr, sct, AF.Ln)
                nc.scalar.activation(triple[:, 2 * NKC:3 * NKC], lr, AF.Exp,
                                     scale=-1.0)
            else:
                rr = ep.tile([128, NKC], F32, tag="rr")
                nc.vector.reciprocal_approx_fast(rr, sct)
                nc.vector.tensor_scalar(triple[:, 2 * NKC:3 * NKC], rr, 1.0,
                                        None, AL.mult)
            st[j]["triple"] = triple

        def s2(j):
            A = st[j].pop("A")
            triple = st[j].pop("triple")
            r16 = triple[:, 2 * NKC:3 * NKC]
            nc.vector.tensor_tensor(triple[:, 0:NKC], A, r16, AL.mult)  # p0
            nc.vector.tensor_tensor(triple[:, NKC:2 * NKC], r16,
                                    triple[:, 0:NKC], AL.subtract)      # nm0
            # masked shifts: [p', nm'] -> outt2, r' -> xr[960:1440]
            outt2 = ep.tile([128, 2 * NKC], F16, tag="outt2")
            nc.vector.tensor_tensor(
                outt2, triple[:, 0:2 * NKC],
                aux_sb[:, j * 3 * NKC:j * 3 * NKC + 2 * NKC], AL.add)
            xr = ep.tile([128, 3 * NKC], F16, tag="xr")
            nc.vector.tensor_tensor(
                xr[:, 2 * NKC:3 * NKC], r16,
                aux_sb[:, j * 3 * NKC + 2 * NKC:(j + 1) * 3 * NKC], AL.add)
            mnp = ep.tile([128, 2 * NKC], F16, tag="mnp")
            nc.vector.tensor_scalar(mnp, outt2, 0.0, None, AL.min)
            # |x| on f16 = clear the sign bit (bitwise AND on the u16 view)
            nc.vector.tensor_scalar(xr[:, 0:2 * NKC].bitcast(mybir.dt.uint16),
                                    outt2[:, :].bitcast(mybir.dt.uint16),
                                    0x7FFF, None, AL.bitwise_and)
            # one exp over [|p'|, |nm'|, r']
            e012 = ep.tile([128, 3 * NKC], F32, tag="e012")
            nc.scalar.activation(e012, xr, AF.Exp, scale=-1.0)
            st[j]["mnp"] = mnp
            st[j]["e012"] = e012

        def s2b(j):
            e012 = st[j].pop("e012")
            # negate the e^{-r'} block so a single ln(1+x) serves all three
            nc.gpsimd.tensor_tensor(e012[:, 2 * NKC:3 * NKC], zero_sb,
                                    e012[:, 2 * NKC:3 * NKC], AL.subtract)
            l012 = ep.tile([128, 3 * NKC], F16, tag="l012")
            nc.scalar.activation(l012, e012, AF.Ln, bias=1.0)
            st[j]["l012"] = l012

        def s3(j):
            mnp = st[j].pop("mnp")
            l012 = st[j].pop("l012")
            f0s = st[j].pop("f0s")
            uv = ep.tile([128, 2 * NKC], F16, tag="uv")
            nc.vector.tensor_tensor(uv, l012[:, 0:2 * NKC], mnp, AL.subtract)
            w1 = ep.tile([128, NKC], F16, tag="w1")
            nc.vector.tensor_tensor(w1, uv[:, 0:NKC], uv[:, NKC:2 * NKC],
                                    AL.add)
            wt = ep.tile([128, NKC], F16, tag="wt")
            nc.vector.tensor_tensor(wt, w1, l012[:, 2 * NKC:3 * NKC],
                                    AL.subtract)
            S = ep.tile([128, NK], F32, tag="S")
            nc.vector.reduce_sum(S, _v(wt[:, :], 0, [[1, NK], [NK, C]]),
                                 axis=mybir.AxisListType.X)
            z = ep.tile([128, NK], F32, tag="z")
            nc.gpsimd.tensor_tensor(z, f0s, S, AL.subtract)
            m1n = ep5.tile([128, NS], F32, tag="m1n")
            nc.vector.tensor_reduce(m1n, _v(z[:, :], 0, [[K, NS], [1, K]]),
                                    axis=mybir.AxisListType.X, op=AL.max,
                                    negate=True)
            zz = ep.tile([128, NK], F32, tag="zz")
            nc.gpsimd.tensor_tensor(zz, z, _v(m1n[:, :], 0, [[1, NS], [0, K]]),
                                    AL.add)
            st[j]["m1n"] = m1n
            st[j]["zz"] = zz

        def s4(j):
            zz = st[j].pop("zz")
            epair = st[j].pop("epair")
            nc.scalar.activation(epair[:, 0:NK], zz, AF.Exp)
            st[j]["epair2"] = epair

        def s5(j):
            epair = st[j].pop("epair2")
            s12 = ep.tile([128, 2 * NS], F32, tag="s12")
            nc.vector.reduce_sum(
                s12, _v(epair[:, :], 0, [[NK, 2], [K, NS], [1, K]]),
                axis=mybir.AxisListType.X)
            st[j]["s12"] = s12

        def s6(j):
            s12 = st[j].pop("s12")
            lse = ep.tile([128, 2 * NS], F32, tag="lse")
            nc.scalar.activation(lse, s12, AF.Ln)
            st[j]["lse"] = lse

        def s7(j):
            lse = st[j].pop("lse")
            m1n = st[j].pop("m1n")
            dt = ep.tile([128, NS], F32, tag="dt")
            nc.gpsimd.tensor_tensor(dt, lse[:, 0:NS], lse[:, NS:2 * NS],
                                    AL.subtract)
            mx = ep.tile([128, NS], F32, tag="mx")
            nc.gpsimd.tensor_tensor(mx, dt, m1n, AL.subtract)
            nc.vector.reduce_sum(acc[:, 2 * j:2 * j + 2],
                                 _v(mx[:, :], 0, [[8, 2], [1, 8]]),
                                 axis=mybir.AxisListType.X)

        stages = [s1, s1b, s2, s2b, s3, s4, s5, s6, s7]
        dma_tile(0)
        nc.sync.dma_start(out=vp_sb, in_=vp_d)
        nc.sync.dma_start(out=q_sb, in_=q_d)
        dma_tile(1)
        nc.sync.dma_start(out=aux_sb, in_=aux_d)
        for i in range(NT + len(stages) - 1):
            if i + 2 < NT:
                dma_tile(i + 2)
            for d, fn in enumerate(stages):
                jj = i - d
                if 0 <= jj < NT:
                    fn(jj)

        nc.sync.dma_start(out=acc_d, in_=acc)

    nc.compile()
    return nc


_CACHE = {}


def _get_program(pix, with_bias):
    key = (pix, with_bias)
    if key not in _CACHE:
        _CACHE[key] = build_program(pix, with_bias)
    return _CACHE[key]


def _prescale_w(W_conv):
    Wr = W_conv.astype(np.float64).reshape(D, K, 10).copy()
    Wr[:, :, 1:4] *= -127.5
    Wr[:, :, 7:10] *= -1.0
    return np.ascontiguousarray(
        Wr.reshape(D, M).astype(ml_dtypes.bfloat16))


def _pack_host(vf, per):
    """vf [per,3] raw 0..255 -> (vp, q, aux) host tensors for one core.

    Pixel local index = j*2048 + s*128 + partition; partition-major packing:
      vp  [128, NT*NS*C]   : vp2 = value-127            at j*48 + s*3 + c
      q   [128, NT*NS*2]   : vp2_{0,1} - 0.5            at j*32 + s*2 + d
      aux [128, NT*3*480]  : (PK,MK,RK) c-blocked       at j*1440 + b*480
                             + c*160 + s*10 + k  (k-broadcast)
    """
    NT = per // 2048
    v = vf.reshape(NT, NS_G, 128, C).transpose(2, 0, 1, 3)  # [128, NT, s, c]
    vp2 = v - 127.0
    vp = np.ascontiguousarray(
        vp2.reshape(128, -1).astype(np.float16))
    q = np.ascontiguousarray(
        (vp2[..., 0:2] - 0.5).reshape(128, -1).astype(np.float16))
    mlow = (v == 0.0)
    mhigh = (v == 255.0)
    pk = KBIG * mhigh
    mk = KBIG * mlow
    rk = KBIG * (mlow | mhigh)
    aux = np.stack([pk, mk, rk], axis=2)            # [128, NT, b, s, c]
    aux = np.transpose(aux, (0, 1, 2, 4, 3))        # [128, NT, b, c, s]
    aux = np.broadcast_to(aux[..., None], aux.shape + (K,))
    aux = np.ascontiguousarray(
        aux.reshape(128, -1).astype(np.float16))
    return vp, q, aux


NS_G = 16


def shard_inputs(x, value, W_conv, b_conv, n_cores=N_CORES):
    B = x.shape[0]
    pix_total = B * x.shape[1] * x.shape[2]
    per = pix_total // n_cores
    xf = np.ascontiguousarray(x.reshape(pix_total, D).astype(np.float32))
    vf = value.reshape(pix_total, C).astype(np.float32)
    w_bf = _prescale_w(np.asarray(W_conv))
    with_bias = bool(np.any(b_conv))
    in_maps = []
    for i in range(n_cores):
        xT = xf[i * per:(i + 1) * per].T.astype(ml_dtypes.bfloat16)
        # tile-contiguous layout [NT*D, 2048] so each tile DMA is one
        # dense block
        xT = np.ascontiguousarray(
            xT.reshape(D, per // 2048, 2048).transpose(1, 0, 2)
        ).reshape(-1, 2048)
        vp, q, aux = _pack_host(vf[i * per:(i + 1) * per], per)
        mm = {"xT": xT, "w": w_bf, "vp": vp, "q": q, "aux": aux}
        if with_bias:
            br = b_conv.astype(np.float64).reshape(K, 10).copy()
            br[:, 1:4] *= -127.5
            br[:, 7:10] *= -1.0
            mm["bias"] = br.reshape(1, M).astype(ml_dtypes.bfloat16)
        in_maps.append(mm)
    return in_maps, with_bias, per


def kernel(x, value, W_conv, b_conv):
    x = np.asarray(x)
    value = np.asarray(value)
    W_conv = np.asarray(W_conv)
    b_conv = np.asarray(b_conv)
    in_maps, with_bias, per = shard_inputs(x, value, W_conv, b_conv)
    nc = _get_program(per, with_bias)
    res = run_bass_kernel_spmd(nc, in_maps, list(range(N_CORES)))
    parts = []
    for i in range(N_CORES):
        acc = res.results[i]["acc"]
        parts.append(acc.astype(np.float64).sum(axis=0).astype(np.float32))
    return np.concatenate(parts)


# revision 22
# speedup vs baseline: 1.0361x; 1.0361x over previous
"""Trainium2 Bass kernel for nn_LogisticMixture (discretized logistic mixture loss).

Contract: kernel(**inputs) takes FULL unsharded numpy inputs
  x      [128, 32, 32, 256] f32
  value  [128, 32, 32, 3]   f32 (integer pixel values 0..255)
  W_conv [256, 100]         f32
  b_conv [100]              f32
and returns the full [128] f32 output (per-image sum of mixture log-probs).

Strategy: pure data parallelism over batch across 8 NeuronCores (16384
pixels/core). Host pre-transposes x to x^T [256, 16384] bf16 per core and
PRESCALES W columns (locs *= -127.5, coeffs *= -1) so the PE emits
epilogue-ready params. Epilogue per 2048-px macro-tile, software-pipelined
8 stages deep so every cross-engine dependency is >=1 tile old:

  A_c   = vp2_c + locs'_c (+ q*coeff' couplings)        [vp2 = value-127]
  sct   = 127.5*ln(1+e^s) + 127.5e-7 ; r = 1/sct        (recip on DVE or
          via ACT exp(-ln(sct)), split per-tile to balance engines)
  p0 = A*r ; nm0 = r - p0  (negated min_in)
  masked-shift: +30000 host tensors kill u/v/L2 terms for y==0/255 exactly
  u+v   = L01 - min(pm',0) with L01 = ln(1+e^{-|pm'|})  (one paired op)
  w     = u + v - ln(1-e^{-r'}) ; S = sum_c w
  mix   = lse_k(logits - S) - lse_k(logits)             -> acc per image

Engine split: DVE does the f16 2x/4x-mode elementwise; ACT all exp/ln
(single natural_log_exp_and_others table); GpSimd (Pool) the coupling
products, psum copies and f32 adds; PE the 1x1-conv matmuls.
"""
import sys
import os

for _p in ("/opt/trn_rl_repo", "/root/.axon_site/_ro/trn_rl_repo"):
    if os.path.isdir(_p) and _p not in sys.path:
        sys.path.append(_p)

import numpy as np
import ml_dtypes

import concourse.bass as bass
import concourse.mybir as mybir
import concourse.tile as tile
from concourse import bacc
from concourse.bass_utils import run_bass_kernel_spmd
import concourse.hw_specs as hw_specs

F32 = mybir.dt.float32
F16 = mybir.dt.float16
BF16 = mybir.dt.bfloat16
AL = mybir.AluOpType
AF = mybir.ActivationFunctionType

N_CORES = 8
D = 256
M = 100          # NUM_MIX * NUM_OUT
K = 10           # mixtures
C = 3            # channels
C2 = float(127.5 * np.exp(-7.0))   # eps folded into scales_t
KBIG = 30000.0                     # masked-shift magnitude (f16-exact)
N_ACT_RECIP = 2                    # tiles whose reciprocal runs on ACT


def _force_single_act_table():
    """All ACT funcs used here (Exp, Ln) live in natural_log_exp_and_others.
    The default chooser flip-flops between exp/ln sets, reloading tables
    (~1.3us each). Empty every other set (keeping dict order so set ids stay
    aligned with act_info.json) so one table load serves the whole kernel."""
    if getattr(hw_specs, "_ant_single_set", False):
        return
    orig = hw_specs.get_activation_tables
    import functools

    @functools.cache
    def patched(arch):
        tabs = dict(orig(arch))
        keep = "natural_log_exp_and_others"
        if keep in tabs:
            tabs = {k: (v if k == keep else set()) for k, v in tabs.items()}
        return tabs

    hw_specs.get_activation_tables = patched
    bacc.get_activation_tables = patched
    hw_specs._ant_single_set = True


def _v(ap0, offset, pattern):
    """AP on a tile's [:, :] AP: keep partition dim, replace free dims
    (stride-0 broadcast dims allowed)."""
    if not isinstance(ap0, bass.AP):
        ap0 = ap0[:, :]
    return bass.AP(tensor=ap0.tensor, offset=ap0.offset + offset,
                   ap=[list(ap0.ap[0])] + [list(p) for p in pattern])


def build_program(pix=16384, with_bias=False):
    """Single-core SPMD program. pix must be a multiple of 2048."""
    TP = 2048                  # pixels per macro-tile (2 images)
    NT = pix // TP             # macro-tiles
    NS = 16                    # 128-px subtiles per macro-tile
    NIMG = pix // 1024
    NKC = NS * K * C           # 480
    NK = NS * K                # 160

    _force_single_act_table()
    nc = bacc.Bacc("TRN2", target_bir_lowering=False, debug=False)

    xT_d = nc.dram_tensor("xT", [NT * D, TP], BF16, kind="ExternalInput").ap()
    w_d = nc.dram_tensor("w", [D, M], BF16, kind="ExternalInput").ap()
    vp_d = nc.dram_tensor("vp", [128, NT * NS * C], F16, kind="ExternalInput").ap()
    q_d = nc.dram_tensor("q", [128, NT * NS * 2], F16, kind="ExternalInput").ap()
    aux_d = nc.dram_tensor("aux", [128, NT * 3 * NKC], F16,
                           kind="ExternalInput").ap()
    if with_bias:
        bias_d = nc.dram_tensor("bias", [1, M], BF16, kind="ExternalInput").ap()
    acc_d = nc.dram_tensor("acc", [128, NIMG], F32, kind="ExternalOutput").ap()

    with tile.TileContext(nc) as tc, \
            tc.tile_pool(name="const", bufs=1) as cpool, \
            tc.tile_pool(name="xin", bufs=3) as xpool, \
            tc.tile_pool(name="ps", bufs=2, space="PSUM") as pspool, \
            tc.tile_pool(name="ep", bufs=3) as ep, \
            tc.tile_pool(name="ep4", bufs=4) as ep4, \
            tc.tile_pool(name="ep5", bufs=5) as ep5:

        # DMA order matters for pipeline fill: w + first x tiles gate the
        # PE; aux is not needed until s2(0), so it goes last.
        w_sb = cpool.tile([128, 2 * M], BF16)
        nc.sync.dma_start(out=w_sb, in_=bass.AP(
            tensor=w_d.tensor, offset=0, ap=[[M, 128], [128 * M, 2], [1, M]]))
        vp_sb = cpool.tile([128, NT * NS * C], F16)
        q_sb = cpool.tile([128, NT * NS * 2], F16)
        aux_sb = cpool.tile([128, NT * 3 * NKC], F16)
        acc = cpool.tile([128, NIMG], F32)
        zero_sb = cpool.tile([128, NKC], F32)
        nc.gpsimd.memset(zero_sb, 0.0)
        if with_bias:
            bias_sb = cpool.tile([1, M], BF16)
            nc.sync.dma_start(out=bias_sb, in_=bias_d)
            ones_sb = cpool.tile([1, 128], BF16)
            nc.vector.memset(ones_sb, 1.0)

        # per-tile state passed between stages
        st = [dict() for _ in range(NT)]

        # AP dim patterns
        #  psum field view for field f0 at offset f: [s(128), k(10), (c)]
        def fview(ps, f, cdim=False):
            dims = [[128, NS], [K, K]] + ([[1, C]] if cdim else [])
            return _v(ps[:, :], f, dims)

        cb_out = [[K, NS], [1, K], [NK, C]]     # c-blocked out iter [s,k,c]

        def dma_tile(j):
            xt0 = xpool.tile([128, TP], BF16, tag="xt0")
            xt1 = xpool.tile([128, TP], BF16, tag="xt1")
            nc.sync.dma_start(out=xt0, in_=xT_d[j * D:j * D + 128, :])
            nc.sync.dma_start(out=xt1, in_=xT_d[j * D + 128:(j + 1) * D, :])
            st[j]["xt"] = (xt0, xt1)

        def s1(j):
            xt0, xt1 = st[j].pop("xt")
            ps = pspool.tile([128, 2048], F32, tag="ps")
            for sub in range(NS):
                o = ps[:, sub * 128:sub * 128 + M]
                if with_bias:
                    nc.tensor.matmul(o, ones_sb[:, :], bias_sb[:, :],
                                     start=True, stop=False)
                nc.tensor.matmul(o, xt0[:, sub * 128:(sub + 1) * 128],
                                 w_sb[:, 0:M], start=not with_bias, stop=False)
                nc.tensor.matmul(o, xt1[:, sub * 128:(sub + 1) * 128],
                                 w_sb[:, M:2 * M], start=False, stop=True)

            # ACT: Es = exp(s_raw) (c-blocked out), Ls = ln(1+Es)
            es = ep.tile([128, NKC], F16, tag="es")
            nc.scalar.activation(_v(es[:, :], 0, cb_out), fview(ps, 4, True),
                                 AF.Exp)
            ls = ep.tile([128, NKC], F16, tag="ls")
            nc.scalar.activation(ls, es, AF.Ln, bias=1.0)
            # ACT: e2t = exp(logits) -> epair[160:320]
            epair = ep5.tile([128, 2 * NK], F32, tag="epair")
            nc.scalar.activation(epair[:, NK:2 * NK], fview(ps, 0), AF.Exp)
            # ACT copies out of PSUM (GpSimd cannot touch PSUM on TRN2):
            # logits f0 -> f0s, coeff fields f7..9 -> cc (coeff-blocked)
            f0s = ep4.tile([128, NK], F32, tag="f0s")
            nc.scalar.activation(f0s, fview(ps, 0), AF.Copy)
            cc = ep.tile([128, NKC], F16, tag="cc")
            nc.scalar.activation(_v(cc[:, :], 0, cb_out), fview(ps, 7, True),
                                 AF.Copy)
            # DVE: A = locs' + vp2 (c-blocked)
            A = ep.tile([128, NKC], F16, tag="A")
            vpb = _v(vp_sb[:, :], j * NS * C, [[C, NS], [0, K], [1, C]])
            nc.vector.tensor_tensor(_v(A[:, :], 0, cb_out), fview(ps, 1, True),
                                    vpb, AL.add)
            # Pool: coupling products + adds into A (all SBUF)
            qb0 = _v(q_sb[:, :], j * NS * 2, [[2, NS], [0, K]])
            qb1 = _v(q_sb[:, :], j * NS * 2 + 1, [[2, NS], [0, K]])
            h0 = ep.tile([128, NK], F16, tag="h0")
            h1 = ep.tile([128, NK], F16, tag="h1")
            h2 = ep.tile([128, NK], F16, tag="h2")
            nc.gpsimd.tensor_tensor(h0, qb0, cc[:, 0:NK], AL.mult)
            nc.gpsimd.tensor_tensor(h1, qb0, cc[:, NK:2 * NK], AL.mult)
            nc.gpsimd.tensor_tensor(h2, qb1, cc[:, 2 * NK:3 * NK], AL.mult)
            nc.gpsimd.tensor_tensor(A[:, NK:2 * NK], A[:, NK:2 * NK], h0,
                                    AL.add)
            nc.gpsimd.tensor_tensor(A[:, 2 * NK:3 * NK], A[:, 2 * NK:3 * NK],
                                    h1, AL.add)
            nc.gpsimd.tensor_tensor(A[:, 2 * NK:3 * NK], A[:, 2 * NK:3 * NK],
                                    h2, AL.add)
            st[j]["es"] = es
            st[j]["ls"] = ls
            st[j]["epair"] = epair
            st[j]["f0s"] = f0s
            st[j]["A"] = A

        def s1b(j):
            ls = st[j].pop("ls")
            sct = ep.tile([128, NKC], F32, tag="sct")
            nc.vector.tensor_scalar(sct, ls, 127.5, C2, AL.mult, AL.add)
            triple = ep.tile([128, 3 * NKC], F16, tag="triple")
            if j < N_ACT_RECIP:
                lr = ep.tile([128, NKC], F32, tag="lr")
                nc.scalar.activation(lr, sct, AF.Ln)
                nc.scalar.activation(triple[:, 2 * NKC:3 * NKC], lr, AF.Exp,
                                     scale=-1.0)
            else:
                rr = ep.tile([128, NKC], F32, tag="rr")
                nc.vector.reciprocal_approx_fast(rr, sct)
                nc.vector.tensor_scalar(triple[:, 2 * NKC:3 * NKC], rr, 1.0,
                                        None, AL.mult)
            st[j]["triple"] = triple

        def s2(j):
            A = st[j].pop("A")
            triple = st[j].pop("triple")
            r16 = triple[:, 2 * NKC:3 * NKC]
            nc.vector.tensor_tensor(triple[:, 0:NKC], A, r16, AL.mult)  # p0
            nc.vector.tensor_tensor(triple[:, NKC:2 * NKC], r16,
                                    triple[:, 0:NKC], AL.subtract)      # nm0
            # masked shifts: [p', nm'] -> outt2, r' -> xr[960:1440]
            outt2 = ep.tile([128, 2 * NKC], F16, tag="outt2")
            nc.vector.tensor_tensor(
                outt2, triple[:, 0:2 * NKC],
                aux_sb[:, j * 3 * NKC:j * 3 * NKC + 2 * NKC], AL.add)
            xr = ep.tile([128, 3 * NKC], F16, tag="xr")
            nc.vector.tensor_tensor(
                xr[:, 2 * NKC:3 * NKC], r16,
                aux_sb[:, j * 3 * NKC + 2 * NKC:(j + 1) * 3 * NKC], AL.add)
            mnp = ep.tile([128, 2 * NKC], F16, tag="mnp")
            nc.vector.tensor_scalar(mnp, outt2, 0.0, None, AL.min)
            # |x| on f16 = clear the sign bit (bitwise AND on the u16 view)
            nc.vector.tensor_scalar(xr[:, 0:2 * NKC].bitcast(mybir.dt.uint16),
                                    outt2[:, :].bitcast(mybir.dt.uint16),
                                    0x7FFF, None, AL.bitwise_and)
            # one exp over [|p'|, |nm'|, r']
            e012 = ep.tile([128, 3 * NKC], F32, tag="e012")
            nc.scalar.activation(e012, xr, AF.Exp, scale=-1.0)
            st[j]["mnp"] = mnp
            st[j]["e012"] = e012

        def s2b(j):
            e012 = st[j].pop("e012")
            # negate the e^{-r'} block so a single ln(1+x) serves all three
            nc.gpsimd.tensor_tensor(e012[:, 2 * NKC:3 * NKC], zero_sb,
                                    e012[:, 2 * NKC:3 * NKC], AL.subtract)
            l012 = ep.tile([128, 3 * NKC], F16, tag="l012")
            nc.scalar.activation(l012, e012, AF.Ln, bias=1.0)
            st[j]["l012"] = l012

        def s3(j):
            mnp = st[j].pop("mnp")
            l012 = st[j].pop("l012")
            f0s = st[j].pop("f0s")
            uv = ep.tile([128, 2 * NKC], F16, tag="uv")
            nc.vector.tensor_tensor(uv, l012[:, 0:2 * NKC], mnp, AL.subtract)
            w1 = ep.tile([128, NKC], F16, tag="w1")
            nc.vector.tensor_tensor(w1, uv[:, 0:NKC], uv[:, NKC:2 * NKC],
                                    AL.add)
            wt = ep.tile([128, NKC], F16, tag="wt")
            nc.vector.tensor_tensor(wt, w1, l012[:, 2 * NKC:3 * NKC],
                                    AL.subtract)
            S = ep.tile([128, NK], F32, tag="S")
            nc.vector.reduce_sum(S, _v(wt[:, :], 0, [[1, NK], [NK, C]]),
                                 axis=mybir.AxisListType.X)
            z = ep.tile([128, NK], F32, tag="z")
            nc.gpsimd.tensor_tensor(z, f0s, S, AL.subtract)
            m1n = ep5.tile([128, NS], F32, tag="m1n")
            nc.vector.tensor_reduce(m1n, _v(z[:, :], 0, [[K, NS], [1, K]]),
                                    axis=mybir.AxisListType.X, op=AL.max,
                                    negate=True)
            zz = ep.tile([128, NK], F32, tag="zz")
            nc.gpsimd.tensor_tensor(zz, z, _v(m1n[:, :], 0, [[1, NS], [0, K]]),
                                    AL.add)
            st[j]["m1n"] = m1n
            st[j]["zz"] = zz

        def s4(j):
            zz = st[j].pop("zz")
            epair = st[j].pop("epair")
            nc.scalar.activation(epair[:, 0:NK], zz, AF.Exp)
            st[j]["epair2"] = epair

        def s5(j):
            epair = st[j].pop("epair2")
            s12 = ep.tile([128, 2 * NS], F32, tag="s12")
            nc.vector.reduce_sum(
                s12, _v(epair[:, :], 0, [[NK, 2], [K, NS], [1, K]]),
                axis=mybir.AxisListType.X)
            st[j]["s12"] = s12

        def s6(j):
            s12 = st[j].pop("s12")
            lse = ep.tile([128, 2 * NS], F32, tag="lse")
            nc.scalar.activation(lse, s12, AF.Ln)
            st[j]["lse"] = lse

        def s7(j):
            lse = st[j].pop("lse")
            m1n = st[j].pop("m1n")
            dt = ep.tile([128, NS], F32, tag="dt")
            nc.gpsimd.tensor_tensor(dt, lse[:, 0:NS], lse[:, NS:2 * NS],
                                    AL.subtract)
            mx = ep.tile([128, NS], F32, tag="mx")
            nc.gpsimd.tensor_tensor(mx, dt, m1n, AL.subtract)
            nc.vector.reduce_sum(acc[:, 2 * j:2 * j + 2],
                                 _v(mx[:, :], 0, [[8, 2], [1, 8]]),
                                 axis=mybir.AxisListType.X)

        stages = [s1, s1b, s2, s2b, s3, s4, s5, s6, s7]
        dma_tile(0)
        nc.sync.dma_start(out=vp_sb, in_=vp_d)
        nc.sync.dma_start(out=q_sb, in_=q_d)
        dma_tile(1)
        nc.sync.dma_start(out=aux_sb, in_=aux_d)
        # deepest stage first: each engine opens the iteration with work on
        # the oldest in-flight tile, whose cross-engine deps resolved in
        # earlier iterations
        for i in range(NT + len(stages) - 1):
            if i + 2 < NT:
                dma_tile(i + 2)
            for d in range(len(stages) - 1, -1, -1):
                jj = i - d
                if 0 <= jj < NT:
                    stages[d](jj)

        nc.sync.dma_start(out=acc_d, in_=acc)

    nc.compile()
    return nc


_CACHE = {}


def _get_program(pix, with_bias):
    key = (pix, with_bias)
    if key not in _CACHE:
        _CACHE[key] = build_program(pix, with_bias)
    return _CACHE[key]


def _prescale_w(W_conv):
    Wr = W_conv.astype(np.float64).reshape(D, K, 10).copy()
    Wr[:, :, 1:4] *= -127.5
    Wr[:, :, 7:10] *= -1.0
    return np.ascontiguousarray(
        Wr.reshape(D, M).astype(ml_dtypes.bfloat16))


def _pack_host(vf, per):
    """vf [per,3] raw 0..255 -> (vp, q, aux) host tensors for one core.

    Pixel local index = j*2048 + s*128 + partition; partition-major packing:
      vp  [128, NT*NS*C]   : vp2 = value-127            at j*48 + s*3 + c
      q   [128, NT*NS*2]   : vp2_{0,1} - 0.5            at j*32 + s*2 + d
      aux [128, NT*3*480]  : (PK,MK,RK) c-blocked       at j*1440 + b*480
                             + c*160 + s*10 + k  (k-broadcast)
    """
    NT = per // 2048
    v = vf.reshape(NT, NS_G, 128, C).transpose(2, 0, 1, 3)  # [128, NT, s, c]
    vp2 = v - 127.0
    vp = np.ascontiguousarray(
        vp2.reshape(128, -1).astype(np.float16))
    q = np.ascontiguousarray(
        (vp2[..., 0:2] - 0.5).reshape(128, -1).astype(np.float16))
    mlow = (v == 0.0)
    mhigh = (v == 255.0)
    pk = KBIG * mhigh
    mk = KBIG * mlow
    rk = KBIG * (mlow | mhigh)
    aux = np.stack([pk, mk, rk], axis=2)            # [128, NT, b, s, c]
    aux = np.transpose(aux, (0, 1, 2, 4, 3))        # [128, NT, b, c, s]
    aux = np.broadcast_to(aux[..., None], aux.shape + (K,))
    aux = np.ascontiguousarray(
        aux.reshape(128, -1).astype(np.float16))
    return vp, q, aux


NS_G = 16


def shard_inputs(x, value, W_conv, b_conv, n_cores=N_CORES):
    B = x.shape[0]
    pix_total = B * x.shape[1] * x.shape[2]
    per = pix_total // n_cores
    xf = np.ascontiguousarray(x.reshape(pix_total, D).astype(np.float32))
    vf = value.reshape(pix_total, C).astype(np.float32)
    w_bf = _prescale_w(np.asarray(W_conv))
    with_bias = bool(np.any(b_conv))
    in_maps = []
    for i in range(n_cores):
        xT = xf[i * per:(i + 1) * per].T.astype(ml_dtypes.bfloat16)
        # tile-contiguous layout [NT*D, 2048] so each tile DMA is one
        # dense block
        xT = np.ascontiguousarray(
            xT.reshape(D, per // 2048, 2048).transpose(1, 0, 2)
        ).reshape(-1, 2048)
        vp, q, aux = _pack_host(vf[i * per:(i + 1) * per], per)
        mm = {"xT": xT, "w": w_bf, "vp": vp, "q": q, "aux": aux}
        if with_bias:
            br = b_conv.astype(np.float64).reshape(K, 10).copy()
            br[:, 1:4] *= -127.5
            br[:, 7:10] *= -1.0
            mm["bias"] = br.reshape(1, M).astype(ml_dtypes.bfloat16)
        in_maps.append(mm)
    return in_maps, with_bias, per


def kernel(x, value, W_conv, b_conv):
    x = np.asarray(x)
    value = np.asarray(value)
    W_conv = np.asarray(W_conv)
    b_conv = np.asarray(b_conv)
    in_maps, with_bias, per = shard_inputs(x, value, W_conv, b_conv)
    nc = _get_program(per, with_bias)
    res = run_bass_kernel_spmd(nc, in_maps, list(range(N_CORES)))
    parts = []
    for i in range(N_CORES):
        acc = res.results[i]["acc"]
        parts.append(acc.astype(np.float64).sum(axis=0).astype(np.float32))
    return np.concatenate(parts)


# revision 28
# speedup vs baseline: 1.2403x; 1.1970x over previous
"""Trainium2 Bass kernel for nn_LogisticMixture (discretized logistic mixture loss).

Contract: kernel(**inputs) takes FULL unsharded numpy inputs
  x      [128, 32, 32, 256] f32
  value  [128, 32, 32, 3]   f32 (integer pixel values 0..255)
  W_conv [256, 100]         f32
  b_conv [100]              f32
and returns the full [128] f32 output (per-image sum of mixture log-probs).

Strategy: pure data parallelism over batch across 8 NeuronCores (16384
pixels/core). Host pre-transposes x to x^T [256, 16384] bf16 per core and
PRESCALES W columns (locs *= -127.5, coeffs *= -1) so the PE emits
epilogue-ready params. Epilogue per 2048-px macro-tile, software-pipelined
8 stages deep so every cross-engine dependency is >=1 tile old:

  A_c   = vp2_c + locs'_c (+ q*coeff' couplings)        [vp2 = value-127]
  sct   = 127.5*ln(1+e^s) + 127.5e-7 ; r = 1/sct        (recip on DVE or
          via ACT exp(-ln(sct)), split per-tile to balance engines)
  p0 = A*r ; nm0 = r - p0  (negated min_in)
  masked-shift: +30000 host tensors kill u/v/L2 terms for y==0/255 exactly
  u+v   = L01 - min(pm',0) with L01 = ln(1+e^{-|pm'|})  (one paired op)
  w     = u + v - ln(1-e^{-r'}) ; S = sum_c w
  mix   = lse_k(logits - S) - lse_k(logits)             -> acc per image

Engine split: DVE does the f16 2x/4x-mode elementwise; ACT all exp/ln
(single natural_log_exp_and_others table); GpSimd (Pool) the coupling
products, psum copies and f32 adds; PE the 1x1-conv matmuls.
"""
import sys
import os

for _p in ("/opt/trn_rl_repo", "/root/.axon_site/_ro/trn_rl_repo"):
    if os.path.isdir(_p) and _p not in sys.path:
        sys.path.append(_p)

import numpy as np
import ml_dtypes

import concourse.bass as bass
import concourse.mybir as mybir
import concourse.tile as tile
from concourse import bacc
from concourse.bass_utils import run_bass_kernel_spmd
import concourse.hw_specs as hw_specs

F32 = mybir.dt.float32
F16 = mybir.dt.float16
BF16 = mybir.dt.bfloat16
AL = mybir.AluOpType
AF = mybir.ActivationFunctionType

N_CORES = 8
D = 256
M = 100          # NUM_MIX * NUM_OUT
K = 10           # mixtures
C = 3            # channels
C2 = float(127.5 * np.exp(-7.0))   # eps folded into scales_t
KBIG = 30000.0                     # masked-shift magnitude (f16-exact)
N_ACT_RECIP = 2                    # tiles whose reciprocal runs on ACT


def _force_single_act_table():
    """All ACT funcs used here (Exp, Ln) live in natural_log_exp_and_others.
    The default chooser flip-flops between exp/ln sets, reloading tables
    (~1.3us each). Empty every other set (keeping dict order so set ids stay
    aligned with act_info.json) so one table load serves the whole kernel."""
    if getattr(hw_specs, "_ant_single_set", False):
        return
    orig = hw_specs.get_activation_tables
    import functools

    @functools.cache
    def patched(arch):
        tabs = dict(orig(arch))
        keep = "natural_log_exp_and_others"
        if keep in tabs:
            tabs = {k: (v if k == keep else set()) for k, v in tabs.items()}
        return tabs

    hw_specs.get_activation_tables = patched
    bacc.get_activation_tables = patched
    hw_specs._ant_single_set = True


def _v(ap0, offset, pattern):
    """AP on a tile's [:, :] AP: keep partition dim, replace free dims
    (stride-0 broadcast dims allowed)."""
    if not isinstance(ap0, bass.AP):
        ap0 = ap0[:, :]
    return bass.AP(tensor=ap0.tensor, offset=ap0.offset + offset,
                   ap=[list(ap0.ap[0])] + [list(p) for p in pattern])


def build_program(pix=16384, with_bias=False):
    """Single-core SPMD program. pix must be a multiple of 2048."""
    TP = 2048                  # pixels per macro-tile (2 images)
    NT = pix // TP             # macro-tiles
    NS = 16                    # 128-px subtiles per macro-tile
    NIMG = pix // 1024
    NKC = NS * K * C           # 480
    NK = NS * K                # 160

    _force_single_act_table()
    nc = bacc.Bacc("TRN2", target_bir_lowering=False, debug=False)

    xT_d = nc.dram_tensor("xT", [NT * D, TP], BF16, kind="ExternalInput").ap()
    w_d = nc.dram_tensor("w", [D, M], BF16, kind="ExternalInput").ap()
    vp_d = nc.dram_tensor("vp", [128, NT * NS * C], F16, kind="ExternalInput").ap()
    q_d = nc.dram_tensor("q", [128, NT * NS * 2], F16, kind="ExternalInput").ap()
    aux_d = nc.dram_tensor("aux", [128, NT * 3 * NKC], F16,
                           kind="ExternalInput").ap()
    if with_bias:
        bias_d = nc.dram_tensor("bias", [1, M], BF16, kind="ExternalInput").ap()
    acc_d = nc.dram_tensor("acc", [128, NIMG], F32, kind="ExternalOutput").ap()

    with tile.TileContext(nc) as tc, \
            tc.tile_pool(name="const", bufs=1) as cpool, \
            tc.tile_pool(name="xin", bufs=3) as xpool, \
            tc.tile_pool(name="ps", bufs=2, space="PSUM") as pspool, \
            tc.tile_pool(name="ep", bufs=3) as ep, \
            tc.tile_pool(name="ep4", bufs=4) as ep4, \
            tc.tile_pool(name="ep5", bufs=5) as ep5:

        # DMA order matters for pipeline fill: w + first x tiles gate the
        # PE; aux is not needed until s2(0), so it goes last.
        w_sb = cpool.tile([128, 2 * M], BF16)
        nc.sync.dma_start(out=w_sb, in_=bass.AP(
            tensor=w_d.tensor, offset=0, ap=[[M, 128], [128 * M, 2], [1, M]]))
        vp_sb = cpool.tile([128, NT * NS * C], F16)
        q_sb = cpool.tile([128, NT * NS * 2], F16)
        aux_sb = cpool.tile([128, NT * 3 * NKC], F16)
        acc = cpool.tile([128, NIMG], F32)
        zero_sb = cpool.tile([128, NKC], F32)
        nc.gpsimd.memset(zero_sb, 0.0)
        if with_bias:
            bias_sb = cpool.tile([1, M], BF16)
            nc.sync.dma_start(out=bias_sb, in_=bias_d)
            ones_sb = cpool.tile([1, 128], BF16)
            nc.vector.memset(ones_sb, 1.0)

        # per-tile state passed between stages
        st = [dict() for _ in range(NT)]

        # AP dim patterns
        #  psum field view for field f0 at offset f: [s(128), k(10), (c)]
        def fview(ps, f, cdim=False):
            dims = [[128, NS], [K, K]] + ([[1, C]] if cdim else [])
            return _v(ps[:, :], f, dims)

# layout note: per-tile epilogue tensors use natural (s,k,c) interleave
        # (index = s*30 + k*3 + c) so every ACT/DVE op reads AND writes packed

        def dma_tile(j):
            xt0 = xpool.tile([128, TP], BF16, tag="xt0")
            xt1 = xpool.tile([128, TP], BF16, tag="xt1")
            nc.sync.dma_start(out=xt0, in_=xT_d[j * D:j * D + 128, :])
            nc.sync.dma_start(out=xt1, in_=xT_d[j * D + 128:(j + 1) * D, :])
            st[j]["xt"] = (xt0, xt1)

        def s1(j):
            xt0, xt1 = st[j].pop("xt")
            ps = pspool.tile([128, 2048], F32, tag="ps")
            for sub in range(NS):
                o = ps[:, sub * 128:sub * 128 + M]
                if with_bias:
                    nc.tensor.matmul(o, ones_sb[:, :], bias_sb[:, :],
                                     start=True, stop=False)
                nc.tensor.matmul(o, xt0[:, sub * 128:(sub + 1) * 128],
                                 w_sb[:, 0:M], start=not with_bias, stop=False)
                nc.tensor.matmul(o, xt1[:, sub * 128:(sub + 1) * 128],
                                 w_sb[:, M:2 * M], start=False, stop=True)

            # ACT: Es = exp(s_raw) packed (s,k,c), Ls = ln(1+Es)
            es = ep.tile([128, NKC], F16, tag="es")
            nc.scalar.activation(es, fview(ps, 4, True), AF.Exp)
            ls = ep.tile([128, NKC], F16, tag="ls")
            nc.scalar.activation(ls, es, AF.Ln, bias=1.0)
            # ACT: e2t = exp(logits) -> epair[160:320]
            epair = ep5.tile([128, 2 * NK], F32, tag="epair")
            nc.scalar.activation(epair[:, NK:2 * NK], fview(ps, 0), AF.Exp)
            # ACT copies out of PSUM (GpSimd cannot touch PSUM on TRN2):
            # logits f0 -> f0s, coeff fields f7..9 -> cc
            f0s = ep4.tile([128, NK], F32, tag="f0s")
            nc.scalar.activation(f0s, fview(ps, 0), AF.Copy)
            cc = ep.tile([128, NKC], F16, tag="cc")
            nc.scalar.activation(cc, fview(ps, 7, True), AF.Copy)
            # DVE: A = locs' + vp2 (packed (s,k,c))
            A = ep.tile([128, NKC], F16, tag="A")
            vpb = _v(vp_sb[:, :], j * NS * C, [[C, NS], [0, K], [1, C]])
            nc.vector.tensor_tensor(A, fview(ps, 1, True), vpb, AL.add)
            # Pool: coupling products + adds into A's channel slices
            qb0 = _v(q_sb[:, :], j * NS * 2, [[2, NS], [0, K]])
            qb1 = _v(q_sb[:, :], j * NS * 2 + 1, [[2, NS], [0, K]])
            ccv = [_v(cc[:, :], c, [[C * K, NS], [C, K]]) for c in range(C)]
            Av = [_v(A[:, :], c, [[C * K, NS], [C, K]]) for c in range(C)]
            h0 = ep.tile([128, NK], F16, tag="h0")
            h1 = ep.tile([128, NK], F16, tag="h1")
            h2 = ep.tile([128, NK], F16, tag="h2")
            hv = [_v(h[:, :], 0, [[K, NS], [1, K]]) for h in (h0, h1, h2)]
            nc.gpsimd.tensor_tensor(hv[0], qb0, ccv[0], AL.mult)
            nc.gpsimd.tensor_tensor(hv[1], qb0, ccv[1], AL.mult)
            nc.gpsimd.tensor_tensor(hv[2], qb1, ccv[2], AL.mult)
            nc.gpsimd.tensor_tensor(Av[1], Av[1], hv[0], AL.add)
            nc.gpsimd.tensor_tensor(Av[2], Av[2], hv[1], AL.add)
            nc.gpsimd.tensor_tensor(Av[2], Av[2], hv[2], AL.add)
            st[j]["es"] = es
            st[j]["ls"] = ls
            st[j]["epair"] = epair
            st[j]["f0s"] = f0s
            st[j]["A"] = A

        def s1b(j):
            ls = st[j].pop("ls")
            sct = ep.tile([128, NKC], F32, tag="sct")
            nc.vector.tensor_scalar(sct, ls, 127.5, C2, AL.mult, AL.add)
            triple = ep.tile([128, 3 * NKC], F16, tag="triple")
            if j < N_ACT_RECIP:
                lr = ep.tile([128, NKC], F32, tag="lr")
                nc.scalar.activation(lr, sct, AF.Ln)
                nc.scalar.activation(triple[:, 2 * NKC:3 * NKC], lr, AF.Exp,
                                     scale=-1.0)
            else:
                rr = ep.tile([128, NKC], F32, tag="rr")
                nc.vector.reciprocal_approx_fast(rr, sct)
                nc.vector.tensor_scalar(triple[:, 2 * NKC:3 * NKC], rr, 1.0,
                                        None, AL.mult)
            st[j]["triple"] = triple

        def s2(j):
            A = st[j].pop("A")
            triple = st[j].pop("triple")
            r16 = triple[:, 2 * NKC:3 * NKC]
            nc.vector.tensor_tensor(triple[:, 0:NKC], A, r16, AL.mult)  # p0
            nc.vector.tensor_tensor(triple[:, NKC:2 * NKC], r16,
                                    triple[:, 0:NKC], AL.subtract)      # nm0
            # masked shifts: [p', nm'] -> outt2, r' -> xr[960:1440]
            outt2 = ep.tile([128, 2 * NKC], F16, tag="outt2")
            nc.vector.tensor_tensor(
                outt2, triple[:, 0:2 * NKC],
                aux_sb[:, j * 3 * NKC:j * 3 * NKC + 2 * NKC], AL.add)
            xr = ep.tile([128, 3 * NKC], F16, tag="xr")
            nc.vector.tensor_tensor(
                xr[:, 2 * NKC:3 * NKC], r16,
                aux_sb[:, j * 3 * NKC + 2 * NKC:(j + 1) * 3 * NKC], AL.add)
            mnp = ep.tile([128, 2 * NKC], F16, tag="mnp")
            nc.vector.tensor_scalar(mnp, outt2, 0.0, None, AL.min)
            # |x| on f16 = clear the sign bit (bitwise AND on the u16 view)
            nc.vector.tensor_scalar(xr[:, 0:2 * NKC].bitcast(mybir.dt.uint16),
                                    outt2[:, :].bitcast(mybir.dt.uint16),
                                    0x7FFF, None, AL.bitwise_and)
            # one exp over [|p'|, |nm'|, r']
            e012 = ep.tile([128, 3 * NKC], F16, tag="e012")
            nc.scalar.activation(e012, xr, AF.Exp, scale=-1.0)
            st[j]["mnp"] = mnp
            st[j]["e012"] = e012

        def s2b(j):
            e012 = st[j].pop("e012")
            # negate the e^{-r'} block so a single ln(1+x) serves all three
            nc.gpsimd.tensor_tensor(e012[:, 2 * NKC:3 * NKC], zero_sb,
                                    e012[:, 2 * NKC:3 * NKC], AL.subtract)
            l012 = ep.tile([128, 3 * NKC], F16, tag="l012")
            nc.scalar.activation(l012, e012, AF.Ln, bias=1.0)
            st[j]["l012"] = l012

        def s3(j):
            mnp = st[j].pop("mnp")
            l012 = st[j].pop("l012")
            f0s = st[j].pop("f0s")
            uv = ep.tile([128, 2 * NKC], F16, tag="uv")
            nc.vector.tensor_tensor(uv, l012[:, 0:2 * NKC], mnp, AL.subtract)
            w1 = ep.tile([128, NKC], F16, tag="w1")
            nc.vector.tensor_tensor(w1, uv[:, 0:NKC], uv[:, NKC:2 * NKC],
                                    AL.add)
            wt = ep.tile([128, NKC], F16, tag="wt")
            nc.vector.tensor_tensor(wt, w1, l012[:, 2 * NKC:3 * NKC],
                                    AL.subtract)
            S = ep.tile([128, NK], F32, tag="S")
            nc.vector.reduce_sum(S, _v(wt[:, :], 0, [[C, NK], [1, C]]),
                                 axis=mybir.AxisListType.X)
            z = ep.tile([128, NK], F32, tag="z")
            nc.gpsimd.tensor_tensor(z, f0s, S, AL.subtract)
            m1n = ep5.tile([128, NS], F32, tag="m1n")
            nc.vector.tensor_reduce(m1n, _v(z[:, :], 0, [[K, NS], [1, K]]),
                                    axis=mybir.AxisListType.X, op=AL.max,
                                    negate=True)
            zz = ep.tile([128, NK], F32, tag="zz")
            nc.gpsimd.tensor_tensor(zz, z, _v(m1n[:, :], 0, [[1, NS], [0, K]]),
                                    AL.add)
            st[j]["m1n"] = m1n
            st[j]["zz"] = zz

        def s4(j):
            zz = st[j].pop("zz")
            epair = st[j].pop("epair")
            nc.scalar.activation(epair[:, 0:NK], zz, AF.Exp)
            st[j]["epair2"] = epair

        def s5(j):
            epair = st[j].pop("epair2")
            s12 = ep.tile([128, 2 * NS], F32, tag="s12")
            nc.vector.reduce_sum(
                s12, _v(epair[:, :], 0, [[NK, 2], [K, NS], [1, K]]),
                axis=mybir.AxisListType.X)
            st[j]["s12"] = s12

        def s6(j):
            s12 = st[j].pop("s12")
            lse = ep.tile([128, 2 * NS], F32, tag="lse")
            nc.scalar.activation(lse, s12, AF.Ln)
            st[j]["lse"] = lse

        def s7(j):
            lse = st[j].pop("lse")
            m1n = st[j].pop("m1n")
            dt = ep.tile([128, NS], F32, tag="dt")
            nc.gpsimd.tensor_tensor(dt, lse[:, 0:NS], lse[:, NS:2 * NS],
                                    AL.subtract)
            mx = ep.tile([128, NS], F32, tag="mx")
            nc.gpsimd.tensor_tensor(mx, dt, m1n, AL.subtract)
            nc.vector.reduce_sum(acc[:, 2 * j:2 * j + 2],
                                 _v(mx[:, :], 0, [[8, 2], [1, 8]]),
                                 axis=mybir.AxisListType.X)

        stages = [s1, s1b, s2, s2b, s3, s4, s5, s6, s7]
        dma_tile(0)
        nc.sync.dma_start(out=vp_sb, in_=vp_d)
        nc.sync.dma_start(out=q_sb, in_=q_d)
        dma_tile(1)
        nc.sync.dma_start(out=aux_sb, in_=aux_d)
        # deepest stage first: each engine opens the iteration with work on
        # the oldest in-flight tile, whose cross-engine deps resolved in
        # earlier iterations
        for i in range(NT + len(stages) - 1):
            if i + 2 < NT:
                dma_tile(i + 2)
            for d in range(len(stages) - 1, -1, -1):
                jj = i - d
                if 0 <= jj < NT:
                    stages[d](jj)

        nc.sync.dma_start(out=acc_d, in_=acc)

    nc.compile()
    return nc


_CACHE = {}


def _get_program(pix, with_bias):
    key = (pix, with_bias)
    if key not in _CACHE:
        _CACHE[key] = build_program(pix, with_bias)
    return _CACHE[key]


def _prescale_w(W_conv):
    Wr = W_conv.astype(np.float64).reshape(D, K, 10).copy()
    Wr[:, :, 1:4] *= -127.5
    Wr[:, :, 7:10] *= -1.0
    return np.ascontiguousarray(
        Wr.reshape(D, M).astype(ml_dtypes.bfloat16))


def _pack_host(vf, per):
    """vf [per,3] raw 0..255 -> (vp, q, aux) host tensors for one core.

    Pixel local index = j*2048 + s*128 + partition; partition-major packing:
      vp  [128, NT*NS*C]   : vp2 = value-127            at j*48 + s*3 + c
      q   [128, NT*NS*2]   : vp2_{0,1} - 0.5            at j*32 + s*2 + d
      aux [128, NT*3*480]  : (PK,MK,RK) blocks, each (s,k,c)-interleaved
                             at j*1440 + b*480 + s*30 + k*3 + c (k-bcast)
    """
    NT = per // 2048
    v = vf.reshape(NT, NS_G, 128, C).transpose(2, 0, 1, 3)  # [128, NT, s, c]
    vp2 = v - 127.0
    vp = np.ascontiguousarray(
        vp2.reshape(128, -1).astype(np.float16))
    q = np.ascontiguousarray(
        (vp2[..., 0:2] - 0.5).reshape(128, -1).astype(np.float16))
    mlow = (v == 0.0)
    mhigh = (v == 255.0)
    pk = KBIG * mhigh
    mk = KBIG * mlow
    rk = KBIG * (mlow | mhigh)
    aux = np.stack([pk, mk, rk], axis=2)            # [128, NT, b, s, c]
    aux = np.broadcast_to(aux[:, :, :, :, None, :],
                          aux.shape[:4] + (K, C))   # [128, NT, b, s, k, c]
    aux = np.ascontiguousarray(
        aux.reshape(128, -1).astype(np.float16))
    return vp, q, aux


NS_G = 16


def shard_inputs(x, value, W_conv, b_conv, n_cores=N_CORES):
    B = x.shape[0]
    pix_total = B * x.shape[1] * x.shape[2]
    per = pix_total // n_cores
    xf = np.ascontiguousarray(x.reshape(pix_total, D).astype(np.float32))
    vf = value.reshape(pix_total, C).astype(np.float32)
    w_bf = _prescale_w(np.asarray(W_conv))
    with_bias = bool(np.any(b_conv))
    in_maps = []
    for i in range(n_cores):
        xT = xf[i * per:(i + 1) * per].T.astype(ml_dtypes.bfloat16)
        # tile-contiguous layout [NT*D, 2048] so each tile DMA is one
        # dense block
        xT = np.ascontiguousarray(
            xT.reshape(D, per // 2048, 2048).transpose(1, 0, 2)
        ).reshape(-1, 2048)
        vp, q, aux = _pack_host(vf[i * per:(i + 1) * per], per)
        mm = {"xT": xT, "w": w_bf, "vp": vp, "q": q, "aux": aux}
        if with_bias:
            br = b_conv.astype(np.float64).reshape(K, 10).copy()
            br[:, 1:4] *= -127.5
            br[:, 7:10] *= -1.0
            mm["bias"] = br.reshape(1, M).astype(ml_dtypes.bfloat16)
        in_maps.append(mm)
    return in_maps, with_bias, per


def kernel(x, value, W_conv, b_conv):
    x = np.asarray(x)
    value = np.asarray(value)
    W_conv = np.asarray(W_conv)
    b_conv = np.asarray(b_conv)
    in_maps, with_bias, per = shard_inputs(x, value, W_conv, b_conv)
    nc = _get_program(per, with_bias)
    res = run_bass_kernel_spmd(nc, in_maps, list(range(N_CORES)))
    parts = []
    for i in range(N_CORES):
        acc = res.results[i]["acc"]
        parts.append(acc.astype(np.float64).sum(axis=0).astype(np.float32))
    return np.concatenate(parts)
